# revision 12
# baseline (speedup 1.0000x reference)
"""Trainium2 Bass kernel for nn_DeforConv_71605694759687 (gather-based).

ResBlock(stride2, 64->128) + DCNv2 (modulated deformable conv) + BN + ReLU.

Sharding (8 cores): (batch b = core//4, H-quarter q = core%4); each core
computes 32 output rows of out[b] end-to-end locally (halo via recompute,
no collectives).

Unlike the tent-expansion design, deformable sampling here uses real
GPSIMD gathers (ap_gather): per 3x3 tap k, the four bilinear corner
values are gathered from the padded feature map at runtime-computed
int16 indices, multiplied by per-corner coefficient maps
(mask * bilinear weights, broadcast from 18 rows to 128 partitions via
DRAM-bounce replication DMAs), and contracted on the PE with the DCN
weights (9 taps x 4 corners accumulating matmuls).

Spatial positions use the "i-order" i = ((y%2)*128 + x)*16 + y//2 so
that the ap_gather 16-partition index wrap, the offset-conv rhs view,
and the output un-permute are all regular strided APs.
"""

import numpy as np
import ml_dtypes
from contextlib import ExitStack

import concourse.bass as bass
import concourse.tile as tile
from concourse import mybir, bacc
from concourse.bass_utils import run_bass_kernel_spmd

F32 = mybir.dt.float32
F32R = mybir.dt.float32r
BF16 = mybir.dt.bfloat16
I16 = mybir.dt.int16
AL = mybir.AluOpType
AF = mybir.ActivationFunctionType

P = 128
EPS = 1e-5
Ci, Co, DG, Cg = 64, 128, 2, 64
H, W = 128, 128          # output spatial (after stride-2)
QROWS = 32               # output rows per core
FR = 40                  # F_pad rows: h0-3 .. h0+34 (+2 zero guard rows)
FC = 134                 # F_pad cols: x in [-3, 130]
NELEM = FR * FC          # 5360 gather elements per partition
F1R, F1C = 40, 130       # feat1: rows h0-4..h0+35, cols [-1,128]
XR, XC = 81, 258         # x_pad: rows 2*h0-9..2*h0+71, cols [-1,256]
S = 4096                 # spatial positions per core (32*128)
MAGIC = 12582912.0       # 1.5 * 2^23, fp32 RNE rounding trick
IDXMAX = 37 * FC + 132   # max legal idx00


def _bf(x):
    return np.ascontiguousarray(np.asarray(x).astype(ml_dtypes.bfloat16))


def _f(x):
    return np.ascontiguousarray(np.asarray(x, dtype=np.float32))


def build_nc():
    nc = bacc.Bacc(None)

    d_x = nc.dram_tensor("x_shard", [Ci, XR, XC], F32R, kind="ExternalInput")
    d_l1 = nc.dram_tensor("lhsT1", [Ci, 9, P], F32R, kind="ExternalInput")
    d_l2 = nc.dram_tensor("lhsT2", [P, 9, P], BF16, kind="ExternalInput")
    d_lsc = nc.dram_tensor("lhsT_sc", [Ci, P], F32R, kind="ExternalInput")
    d_loff = nc.dram_tensor("lhsT_off", [P, 9, 54], BF16, kind="ExternalInput")
    d_ldcn = nc.dram_tensor("lhsT_dcn", [P, 9, P], BF16, kind="ExternalInput")
    d_cst = nc.dram_tensor("consts", [P, 12], F32, kind="ExternalInput")
    d_yadd = nc.dram_tensor("y_add", [18, S], F32, kind="ExternalInput")
    d_xadd = nc.dram_tensor("x_add", [18, S], F32, kind="ExternalInput")
    d_rm1 = nc.dram_tensor("rowmask1", [P, F1R], F32, kind="ExternalInput")
    d_rmf = nc.dram_tensor("rowmaskF", [P, FR], F32, kind="ExternalInput")
    d_out = nc.dram_tensor("out", [P, QROWS, W], F32, kind="ExternalOutput")

    d_cm = nc.dram_tensor("cmaps", [4, 18, S], BF16, kind="Internal")
    d_iw = nc.dram_tensor("idxw", [18, 16, 256], I16, kind="Internal")

    with tile.TileContext(nc) as tc, ExitStack() as ctx:
        singles = ctx.enter_context(tc.tile_pool(name="singles", bufs=1))

        fpadA = singles.tile([P, FR, FC], F32)     # gather source, col c <-> x-3
        ldcn = singles.tile([P, 9, P], BF16)
        cst = singles.tile([P, 12], F32)
        nc.sync.dma_start(out=ldcn[:], in_=d_ldcn[:])
        nc.sync.dma_start(out=cst[:], in_=d_cst[:])

        inv1, beta1 = cst[:, 0:1], cst[:, 1:2]
        inv2, beta2 = cst[:, 2:3], cst[:, 3:4]
        inv3, beta3 = cst[:, 4:5], cst[:, 5:6]

        nc.vector.memset(fpadA[:, :, 0:3], 0.0)
        nc.vector.memset(fpadA[:, :, FC - 3:FC], 0.0)
        nc.vector.memset(fpadA[:, 38:40, :], 0.0)

        # ================= Phase A: ResBlock =================
        with tc.tile_pool(name="ph_a", bufs=1) as pa, \
             tc.tile_pool(name="psum_a", bufs=2, space="PSUM") as psa:
            x_pad = pa.tile([Ci, XR, XC], F32R)
            feat1 = pa.tile([P, F1R, F1C], BF16)
            l1 = pa.tile([Ci, 9, P], F32R)
            l2 = pa.tile([P, 9, P], BF16)
            lsc = pa.tile([Ci, P], F32R)
            rm1 = pa.tile([P, F1R], F32)
            rmf = pa.tile([P, FR], F32)
            for i in range(8):
                r0, r1 = (XR * i) // 8, (XR * (i + 1)) // 8
                nc.sync.dma_start(out=x_pad[:, r0:r1, :],
                                  in_=d_x[:, r0:r1, :])
            for t, dref in ((l1, d_l1), (l2, d_l2),
                            (lsc, d_lsc), (rm1, d_rm1), (rmf, d_rmf)):
                nc.sync.dma_start(out=t[:], in_=dref[:])

            nc.vector.memset(feat1[:, :, 0:1], 0.0)
            nc.vector.memset(feat1[:, :, F1C - 1:F1C], 0.0)

            # conv1 3x3 s2 + bn1 + relu -> feat1 (bf16)
            for cki in range(10):
                r0 = cki * 4
                ps = psa.tile([P, 4, W], F32)
                for t in range(9):
                    ty, tx = t // 3, t % 3
                    rhs = x_pad[:, 2 * r0 + ty: 2 * r0 + ty + 7: 2,
                                tx: tx + 2 * W - 1: 2]
                    nc.tensor.matmul(ps[:], l1[:, t, :], rhs,
                                     start=(t == 0), stop=(t == 8))
                nc.scalar.activation(feat1[:, r0:r0 + 4, 1:1 + W], ps[:],
                                     AF.Relu, bias=beta1, scale=inv1)
            for ms in range(4):
                r0, r1 = ms * 10, (ms + 1) * 10
                nc.vector.tensor_tensor(
                    feat1[:, r0:r1], feat1[:, r0:r1],
                    rm1[:, r0:r1, None].to_broadcast((P, 10, F1C)), AL.mult)

            # conv2 3x3 s1 (+ folded shortcut) + bn + relu -> fpadA rows 0..37
            for cki in range(10):
                r0 = cki * 4
                nrow = min(4, 38 - r0)
                ps = psa.tile([P, 4, W], F32, tag="ps2")
                for t in range(9):
                    ty, tx = t // 3, t % 3
                    rhs = feat1[:, r0 + ty: r0 + ty + nrow, tx: tx + W]
                    nc.tensor.matmul(ps[:, :nrow], l2[:, t, :], rhs,
                                     start=(t == 0), stop=False)
                rhs_sc = x_pad[:, 2 * r0 + 3: 2 * r0 + 2 + 2 * nrow: 2,
                               1: 2 * W: 2]
                nc.tensor.matmul(ps[:, :nrow], lsc[:], rhs_sc,
                                 start=False, stop=True)
                nc.scalar.activation(fpadA[:, r0:r0 + nrow, 3:3 + W],
                                     ps[:, :nrow], AF.Relu,
                                     bias=beta2, scale=inv2)
            nc.vector.tensor_tensor(
                fpadA[:, 0:38], fpadA[:, 0:38],
                rmf[:, 0:38, None].to_broadcast((P, 38, FC)), AL.mult)

        # ================= Phase B: offsets -> idx + coeff maps =================
        # All per-(k,d) quantities live on partitions 0..17 with the quantity
        # index in the free dim (engines cannot cross partition bases).
        # Processed in 4 chunks of 1024 spatial positions (2 om blocks each).
        with tc.tile_pool(name="ph_b", bufs=1) as pb, \
             tc.tile_pool(name="ph_b_q", bufs=2) as pbq, \
             tc.tile_pool(name="ph_b_tmp", bufs=2) as pbt, \
             tc.tile_pool(name="psum_b", bufs=2, space="PSUM") as psb:
            fpadB = pb.tile([P, FR, FC], BF16)
            loff = pb.tile([P, 9, 54], BF16)
            yadd = pb.tile([18, S], F32)
            xadd = pb.tile([18, S], F32)
            idx16 = pb.tile([18, 16, 256], I16)   # wrapped (p, j) layout
            nc.sync.dma_start(out=loff[:], in_=d_loff[:])
            nc.sync.dma_start(out=yadd[:], in_=d_yadd[:])
            nc.sync.dma_start(out=xadd[:], in_=d_xadd[:])
            nc.vector.tensor_copy(out=fpadB[:], in_=fpadA[:])

            for ch in range(4):
                # q_in rows 0..17, free: [quant, 1024]
                q_in = pbq.tile([18, 3, 1024], F32, tag="q_in")
                for cb2 in range(2):
                    cki = 2 * ch + cb2
                    # offset conv: out channels (quant*18 + 2k+d); i-order.
                    # block cki covers i in [512cki, 512cki+512):
                    # y = 2a + cki//4, x = 32*(cki%4) + j';
                    # rhs rows y+2+ty, cols x+2+tx, (j' outer, a inner)
                    b2, xq = cki // 4, 32 * (cki % 4)
                    ps = psb.tile([54, 512], F32)
                    for t in range(9):
                        ty, tx = t // 3, t % 3
                        rhs = fpadB[:, b2 + 2 + ty: b2 + 2 + ty + 32: 2,
                                    xq + 2 + tx: xq + 2 + tx + 32]
                        nc.tensor.matmul(
                            ps[:].rearrange("p (j a) -> p j a", j=32),
                            loff[:, t, :],
                            rhs.rearrange("p a j -> p j a"),
                            start=(t == 0), stop=(t == 8))
                    om_sb = pbt.tile([54, 512], F32, tag="om_sb")
                    nc.scalar.copy(om_sb[:], ps[:])
                    for q in range(3):
                        nc.gpsimd.dma_start(
                            out=q_in[:, q, cb2 * 512:(cb2 + 1) * 512],
                            in_=om_sb[q * 18:(q + 1) * 18, :])

                qd = pbt.tile([18, 7, 1024], F32, tag="qd")
                qb = pbt.tile([18, 5, 1024], BF16, tag="qb")
                qcc = pbt.tile([18, 4, 1024], BF16, tag="qcc")
                sl = slice(ch * 1024, (ch + 1) * 1024)
                dy, dx, mm = q_in[:, 0, :], q_in[:, 1, :], q_in[:, 2, :]
                y_, x_ = qd[:, 0, :], qd[:, 1, :]
                t1, t2 = qd[:, 2, :], qd[:, 3, :]
                y0, x0 = qd[:, 4, :], qd[:, 5, :]
                idxf = qd[:, 6, :]
                wy, wx, m_ = qb[:, 0, :], qb[:, 1, :], qb[:, 2, :]
                u_, t_ = qb[:, 3, :], qb[:, 4, :]

                nc.vector.scalar_tensor_tensor(
                    y_, dy, cst[0:18, 6:7], yadd[:, sl], AL.add, AL.add)
                nc.vector.scalar_tensor_tensor(
                    x_, dx, cst[0:18, 7:8], xadd[:, sl], AL.add, AL.add)
                # y' = y - 0.5 (folded into yadd); y0 = RNE(y') = floor(y)
                # except at exact-integer y, where wy=1 keeps bilinear exact.
                nc.scalar.activation(t1, y_, AF.Identity, bias=cst[0:18, 9:10])
                nc.scalar.activation(y0, t1, AF.Identity, bias=cst[0:18, 10:11])
                nc.scalar.activation(t2, x_, AF.Identity, bias=cst[0:18, 9:10])
                nc.scalar.activation(x0, t2, AF.Identity, bias=cst[0:18, 10:11])
                nc.vector.scalar_tensor_tensor(wy, y_, 0.5, y0,
                                               AL.add, AL.subtract)
                nc.vector.scalar_tensor_tensor(wx, x_, 0.5, x0,
                                               AL.add, AL.subtract)
                nc.vector.scalar_tensor_tensor(idxf, y0, float(FC), x0,
                                               AL.mult, AL.add)
                nc.vector.tensor_scalar(idxf, idxf, float(IDXMAX), 0.0,
                                        AL.min, AL.max)
                nc.scalar.copy(
                    out=idx16[:, :, ch * 64:(ch + 1) * 64]
                        .rearrange("p q j -> p j q"),
                    in_=idxf)

                nc.scalar.activation(m_, mm, AF.Sigmoid, bias=cst[0:18, 8:9])
                nc.vector.tensor_tensor(u_, m_, wy, AL.mult)
                nc.vector.tensor_tensor(t_, m_, u_, AL.subtract)
                nc.vector.tensor_tensor(qcc[:, 3, :], u_, wx, AL.mult)
                nc.vector.tensor_tensor(qcc[:, 2, :], u_, qcc[:, 3, :],
                                        AL.subtract)
                nc.vector.tensor_tensor(qcc[:, 1, :], t_, wx, AL.mult)
                nc.vector.tensor_tensor(qcc[:, 0, :], t_, qcc[:, 1, :],
                                        AL.subtract)
                for j4 in range(4):
                    nc.sync.dma_start(out=d_cm[j4, :, sl],
                                      in_=qcc[:, j4, :])
                nc.sync.dma_start(
                    out=d_iw[:, :, ch * 64:(ch + 1) * 64],
                    in_=idx16[:, :, ch * 64:(ch + 1) * 64])

        # ================= Phase C: gather + hadamard + einsum =================
        with tc.tile_pool(name="idxp", bufs=1) as idxp, \
             tc.tile_pool(name="cbp", bufs=3) as cbp, \
             tc.tile_pool(name="vp", bufs=1) as vp, \
             tc.tile_pool(name="pp", bufs=3) as ppool, \
             tc.tile_pool(name="psum_c", bufs=1, space="PSUM") as psc, \
             tc.tile_pool(name="outp", bufs=1) as outp:
            pos = psc.tile([P, S], F32)
            fflat = fpadA[:].rearrange("p a b -> p (a b)")
            # idx layout [p, k, hf, corner, 128]: a half's 4 corner lists are
            # contiguous so one 4-corner, 8192-idx gather covers them (the
            # cost model charges max(out_free, num_elems) per gather, so
            # batch the output well past the 5360-element source scan).
            idxall = idxp.tile([P, 9, 2, 4, 128], I16)
            HS = S // 2
            for hf in range(2):
                for k in range(9):
                    for dd in range(2):
                        nc.sync.dma_start(
                            out=idxall[dd * 64:(dd + 1) * 64, k, hf, 0, :],
                            in_=d_iw[2 * k + dd, None, :, hf * 128:(hf + 1) * 128]
                                .to_broadcast([4, 16, 128]))
                for k in range(9):
                    for sl4, ofs in ((1, 1), (2, FC), (3, FC + 1)):
                        nc.vector.tensor_scalar_add(
                            idxall[:, k, hf, sl4, :],
                            idxall[:, k, hf, 0, :], ofs)
                for k in range(9):
                    v = vp.tile([P, 2 * S], F32, tag=f"v{hf}")
                    nc.gpsimd.ap_gather(
                        out_ap=v[:], in_ap=fflat,
                        idxs_ap=idxall[:, k, hf].rearrange("p a b -> p (a b)"),
                        channels=P, num_elems=NELEM, d=1, num_idxs=2 * S)
                    for j4 in range(4):
                        cb = cbp.tile([P, HS], BF16, tag="cb")
                        nc.sync.dma_start(
                            out=cb[:],
                            in_=d_cm[j4, 2 * k: 2 * k + 2, None,
                                     hf * HS:(hf + 1) * HS]
                                .to_broadcast([2, 64, HS]))
                        pt = ppool.tile([P, HS], BF16, tag="pt")
                        nc.vector.tensor_tensor(
                            pt[:], v[:, j4 * HS:(j4 + 1) * HS], cb[:], AL.mult)
                        for b4 in range(4):
                            nc.tensor.matmul(
                                pos[:, hf * HS + b4 * 512:
                                    hf * HS + (b4 + 1) * 512],
                                ldcn[:, k, :],
                                pt[:, b4 * 512:(b4 + 1) * 512],
                                start=(k == 0 and j4 == 0),
                                stop=(k == 8 and j4 == 3))

            # out stage: bn3 + relu, un-permute i-order -> (y, x)
            ob = outp.tile([P, S], F32)
            obp = ob[:].rearrange("p (a b x) -> p b x a", a=16, b=2)
            posp = pos[:].rearrange("p (b x a) -> p b x a", b=2, x=128)
            for oh in range(2):
                nc.scalar.activation(obp[:, oh], posp[:, oh],
                                     AF.Relu, bias=beta3, scale=inv3)
            nc.sync.dma_start(out=d_out[:],
                              in_=ob[:].rearrange("p (y x) -> p y x", y=QROWS))

    nc.compile()
    return nc


_CACHE = {}


def _prep(inputs):
    f = {k: _f(v) for k, v in inputs.items()}
    inv1 = f['g1'] / np.sqrt(f['v1'] + EPS)
    beta1 = f['b1'] - f['m1'] * inv1
    inv2 = f['g2'] / np.sqrt(f['v2'] + EPS)
    beta2 = f['b2'] - f['m2'] * inv2
    invd = f['gd'] / np.sqrt(f['vd'] + EPS)
    betad = f['bd'] - f['md'] * invd
    inv3 = f['g3'] / np.sqrt(f['v3'] + EPS)
    beta3 = f['b3'] - f['m3'] * inv3

    lhsT1 = np.transpose(f['w1'], (1, 2, 3, 0)).reshape(Ci, 9, P)
    lhsT2 = np.transpose(f['w2'], (1, 2, 3, 0)).reshape(P, 9, P)
    wd = f['wd'][:, :, 0, 0] * (invd / inv2)[:, None]
    lhsT_sc = np.ascontiguousarray(wd.T)

    # offset conv rows: quant*18 + k*2 + d  <-  orig quant*18 + d*9 + k
    perm = np.zeros(54, dtype=np.int64)
    for quant in range(3):
        for kk in range(9):
            for dd in range(2):
                perm[quant * 18 + kk * 2 + dd] = quant * 18 + dd * 9 + kk
    ow = f['off_w'][perm]
    obias = f['off_b'][perm]
    lhsT_off = np.transpose(ow, (1, 2, 3, 0)).reshape(P, 9, 54)

    wr = f['dcn_w'].reshape(Co, DG, Cg, 9)
    lhsT_dcn = np.transpose(wr, (1, 2, 3, 0)).reshape(P, 9, Co)

    cst = np.zeros((P, 12), dtype=np.float32)
    cst[:, 9], cst[:, 10] = MAGIC, -MAGIC
    cst[:, 0], cst[:, 1] = inv1, beta1
    cst[:, 2], cst[:, 3] = inv2, beta2 + betad
    cst[:, 4], cst[:, 5] = inv3, beta3 + inv3 * f['dcn_b']
    for kd in range(18):
        cst[kd, 6] = obias[0 * 18 + kd]   # dy bias
        cst[kd, 7] = obias[1 * 18 + kd]   # dx bias
        cst[kd, 8] = obias[2 * 18 + kd]   # mask bias

    # i-order position constants: i = ((y%2)*128 + x)*16 + y//2
    ii = np.arange(S)
    aa = ii % 16
    cc = ii // 16
    bb2 = cc // 128
    xx = cc % 128
    yloc = 2 * aa + bb2
    y_add = np.zeros((18, S), dtype=np.float32)
    x_add = np.zeros((18, S), dtype=np.float32)
    for kk in range(9):
        for dd in range(2):
            kd = 2 * kk + dd
            y_add[kd] = yloc + (kk // 3) + 1.5
            x_add[kd] = xx + (kk % 3) + 1.5

    return dict(
        lhsT1=_f(lhsT1), lhsT2=_bf(lhsT2), lhsT_sc=_f(lhsT_sc),
        lhsT_off=_bf(lhsT_off), lhsT_dcn=_bf(lhsT_dcn),
        consts=_f(cst), y_add=_f(y_add), x_add=_f(x_add), x=f['x'])


def kernel(**inputs):
    cfg = _prep(inputs)
    x = cfg.pop('x')
    B = x.shape[0]

    if 'nc' not in _CACHE:
        _CACHE['nc'] = build_nc()
    nc = _CACHE['nc']

    in_maps = []
    for cid in range(8):
        b, q = cid // 4, cid % 4
        h0 = 32 * q
        xp = np.zeros((Ci, XR, XC), dtype=np.float32)
        r_lo = 2 * h0 - 9
        s_lo, s_hi = max(r_lo, 0), min(2 * h0 + 72, 256)
        xp[:, s_lo - r_lo: s_hi - r_lo, 1:257] = x[b, :, s_lo:s_hi, :]
        rm1 = np.zeros((P, F1R), dtype=np.float32)
        for f1 in range(F1R):
            rm1[:, f1] = 1.0 if 0 <= h0 - 4 + f1 < H else 0.0
        rmf = np.zeros((P, FR), dtype=np.float32)
        for f2 in range(38):
            rmf[:, f2] = 1.0 if 0 <= h0 - 3 + f2 < H else 0.0
        m = dict(cfg)
        m['x_shard'] = np.ascontiguousarray(xp)
        m['rowmask1'] = rm1
        m['rowmaskF'] = rmf
        in_maps.append(m)

    res = run_bass_kernel_spmd(nc, in_maps, core_ids=list(range(8)))
    out = np.zeros((B, Co, H, W), dtype=np.float32)
    for cid in range(8):
        b, q = cid // 4, cid % 4
        out[b, :, 32 * q:32 * q + 32, :] = res.results[cid]['out']
    return out


# revision 13
# speedup vs baseline: 1.3494x; 1.3494x over previous
"""Trainium2 Bass kernel for nn_DeforConv_71605694759687 (gather-based).

ResBlock(stride2, 64->128) + DCNv2 (modulated deformable conv) + BN + ReLU.

Sharding (8 cores): (batch b = core//4, H-quarter q = core%4); each core
computes 32 output rows of out[b] end-to-end locally (halo via recompute,
no collectives).

Unlike the tent-expansion design, deformable sampling here uses real
GPSIMD gathers (ap_gather): per 3x3 tap k, the four bilinear corner
values are gathered from the padded feature map at runtime-computed
int16 indices, multiplied by per-corner coefficient maps
(mask * bilinear weights, broadcast from 18 rows to 128 partitions via
DRAM-bounce replication DMAs), and contracted on the PE with the DCN
weights (9 taps x 4 corners accumulating matmuls).

Spatial positions use the "i-order" i = ((y%2)*128 + x)*16 + y//2 so
that the ap_gather 16-partition index wrap, the offset-conv rhs view,
and the output un-permute are all regular strided APs.
"""

import numpy as np
import ml_dtypes
from contextlib import ExitStack

import concourse.bass as bass
import concourse.tile as tile
from concourse import mybir, bacc
from concourse.bass_utils import run_bass_kernel_spmd

F32 = mybir.dt.float32
F32R = mybir.dt.float32r
BF16 = mybir.dt.bfloat16
I16 = mybir.dt.int16
AL = mybir.AluOpType
AF = mybir.ActivationFunctionType

P = 128
EPS = 1e-5
Ci, Co, DG, Cg = 64, 128, 2, 64
H, W = 128, 128          # output spatial (after stride-2)
QROWS = 32               # output rows per core
FR = 40                  # F_pad rows: h0-3 .. h0+34 (+2 zero guard rows)
FC = 134                 # F_pad cols: x in [-3, 130]
NELEM = FR * FC          # 5360 gather elements per partition
F1R, F1C = 40, 130       # feat1: rows h0-4..h0+35, cols [-1,128]
XR, XC = 81, 258         # x_pad: rows 2*h0-9..2*h0+71, cols [-1,256]
S = 4096                 # spatial positions per core (32*128)
MAGIC = 12582912.0       # 1.5 * 2^23, fp32 RNE rounding trick
IDXMAX = 37 * FC + 132   # max legal idx00


def _bf(x):
    return np.ascontiguousarray(np.asarray(x).astype(ml_dtypes.bfloat16))


def _f(x):
    return np.ascontiguousarray(np.asarray(x, dtype=np.float32))


def build_nc():
    nc = bacc.Bacc(None)

    d_x = nc.dram_tensor("x_shard", [Ci, XR, XC], F32R, kind="ExternalInput")
    d_l1 = nc.dram_tensor("lhsT1", [Ci, 9, P], F32R, kind="ExternalInput")
    d_l2 = nc.dram_tensor("lhsT2", [P, 9, P], BF16, kind="ExternalInput")
    d_lsc = nc.dram_tensor("lhsT_sc", [Ci, P], F32R, kind="ExternalInput")
    d_loff = nc.dram_tensor("lhsT_off", [P, 9, 54], BF16, kind="ExternalInput")
    d_ldcn = nc.dram_tensor("lhsT_dcn", [P, 9, P], BF16, kind="ExternalInput")
    d_cst = nc.dram_tensor("consts", [P, 12], F32, kind="ExternalInput")
    d_yadd = nc.dram_tensor("y_add", [18, S], F32, kind="ExternalInput")
    d_xadd = nc.dram_tensor("x_add", [18, S], F32, kind="ExternalInput")
    d_rm1 = nc.dram_tensor("rowmask1", [P, F1R], F32, kind="ExternalInput")
    d_rmf = nc.dram_tensor("rowmaskF", [P, FR], F32, kind="ExternalInput")
    d_out = nc.dram_tensor("out", [P, QROWS, W], F32, kind="ExternalOutput")

    d_cm = nc.dram_tensor("cmaps", [4, 18, S], BF16, kind="Internal")
    d_iw = nc.dram_tensor("idxw", [18, 16, 256], I16, kind="Internal")

    with tile.TileContext(nc) as tc, ExitStack() as ctx:
        singles = ctx.enter_context(tc.tile_pool(name="singles", bufs=1))

        fpadA = singles.tile([P, FR, FC], F32)     # gather source, col c <-> x-3
        ldcn = singles.tile([P, 9, P], BF16)
        cst = singles.tile([P, 12], F32)
        nc.sync.dma_start(out=ldcn[:], in_=d_ldcn[:])
        nc.sync.dma_start(out=cst[:], in_=d_cst[:])

        inv1, beta1 = cst[:, 0:1], cst[:, 1:2]
        inv2, beta2 = cst[:, 2:3], cst[:, 3:4]
        inv3, beta3 = cst[:, 4:5], cst[:, 5:6]

        nc.vector.memset(fpadA[:, :, 0:3], 0.0)
        nc.vector.memset(fpadA[:, :, FC - 3:FC], 0.0)
        nc.vector.memset(fpadA[:, 38:40, :], 0.0)

        # ================= Phase A: ResBlock =================
        with tc.tile_pool(name="ph_a", bufs=1) as pa, \
             tc.tile_pool(name="psum_a", bufs=2, space="PSUM") as psa:
            x_pad = pa.tile([Ci, XR, XC], F32R)
            feat1 = pa.tile([P, F1R, F1C], BF16)
            l1 = pa.tile([Ci, 9, P], F32R)
            l2 = pa.tile([P, 9, P], BF16)
            lsc = pa.tile([Ci, P], F32R)
            rm1 = pa.tile([P, F1R], F32)
            rmf = pa.tile([P, FR], F32)
            for i in range(8):
                r0, r1 = (XR * i) // 8, (XR * (i + 1)) // 8
                nc.sync.dma_start(out=x_pad[:, r0:r1, :],
                                  in_=d_x[:, r0:r1, :])
            for t, dref in ((l1, d_l1), (l2, d_l2),
                            (lsc, d_lsc), (rm1, d_rm1), (rmf, d_rmf)):
                nc.sync.dma_start(out=t[:], in_=dref[:])

            nc.vector.memset(feat1[:, :, 0:1], 0.0)
            nc.vector.memset(feat1[:, :, F1C - 1:F1C], 0.0)

            # conv1 3x3 s2 + bn1 + relu -> feat1 (bf16)
            for cki in range(10):
                r0 = cki * 4
                ps = psa.tile([P, 4, W], F32)
                for t in range(9):
                    ty, tx = t // 3, t % 3
                    rhs = x_pad[:, 2 * r0 + ty: 2 * r0 + ty + 7: 2,
                                tx: tx + 2 * W - 1: 2]
                    nc.tensor.matmul(ps[:], l1[:, t, :], rhs,
                                     start=(t == 0), stop=(t == 8))
                nc.scalar.activation(feat1[:, r0:r0 + 4, 1:1 + W], ps[:],
                                     AF.Relu, bias=beta1, scale=inv1)
            for ms in range(4):
                r0, r1 = ms * 10, (ms + 1) * 10
                nc.vector.tensor_tensor(
                    feat1[:, r0:r1], feat1[:, r0:r1],
                    rm1[:, r0:r1, None].to_broadcast((P, 10, F1C)), AL.mult)

            # conv2 3x3 s1 (+ folded shortcut) + bn + relu -> fpadA rows 0..37
            for cki in range(10):
                r0 = cki * 4
                nrow = min(4, 38 - r0)
                ps = psa.tile([P, 4, W], F32, tag="ps2")
                for t in range(9):
                    ty, tx = t // 3, t % 3
                    rhs = feat1[:, r0 + ty: r0 + ty + nrow, tx: tx + W]
                    nc.tensor.matmul(ps[:, :nrow], l2[:, t, :], rhs,
                                     start=(t == 0), stop=False)
                rhs_sc = x_pad[:, 2 * r0 + 3: 2 * r0 + 2 + 2 * nrow: 2,
                               1: 2 * W: 2]
                nc.tensor.matmul(ps[:, :nrow], lsc[:], rhs_sc,
                                 start=False, stop=True)
                nc.scalar.activation(fpadA[:, r0:r0 + nrow, 3:3 + W],
                                     ps[:, :nrow], AF.Relu,
                                     bias=beta2, scale=inv2)
            nc.vector.tensor_tensor(
                fpadA[:, 0:38], fpadA[:, 0:38],
                rmf[:, 0:38, None].to_broadcast((P, 38, FC)), AL.mult)

        # ================= Phase B: offsets -> idx + coeff maps =================
        # All per-(k,d) quantities live on partitions 0..17 with the quantity
        # index in the free dim (engines cannot cross partition bases).
        # Processed in 4 chunks of 1024 spatial positions (2 om blocks each).
        with tc.tile_pool(name="ph_b", bufs=1) as pb, \
             tc.tile_pool(name="ph_b_q", bufs=2) as pbq, \
             tc.tile_pool(name="ph_b_tmp", bufs=2) as pbt, \
             tc.tile_pool(name="psum_b", bufs=2, space="PSUM") as psb:
            fpadB = pb.tile([P, FR, FC], BF16)
            loff = pb.tile([P, 9, 54], BF16)
            yadd = pb.tile([18, S], F32)
            xadd = pb.tile([18, S], F32)
            idx16 = pb.tile([18, 16, 256], I16)   # wrapped (p, j) layout
            nc.sync.dma_start(out=loff[:], in_=d_loff[:])
            nc.sync.dma_start(out=yadd[:], in_=d_yadd[:])
            nc.sync.dma_start(out=xadd[:], in_=d_xadd[:])
            nc.vector.tensor_copy(out=fpadB[:], in_=fpadA[:])

            for ch in range(4):
                # q_in rows 0..17, free: [quant, 1024]
                q_in = pbq.tile([18, 3, 1024], F32, tag="q_in")
                for cb2 in range(2):
                    cki = 2 * ch + cb2
                    # offset conv: out channels (quant*18 + 2k+d); i-order.
                    # block cki covers i in [512cki, 512cki+512):
                    # y = 2a + cki//4, x = 32*(cki%4) + j';
                    # rhs rows y+2+ty, cols x+2+tx, (j' outer, a inner)
                    b2, xq = cki // 4, 32 * (cki % 4)
                    ps = psb.tile([54, 512], F32)
                    for t in range(9):
                        ty, tx = t // 3, t % 3
                        rhs = fpadB[:, b2 + 2 + ty: b2 + 2 + ty + 32: 2,
                                    xq + 2 + tx: xq + 2 + tx + 32]
                        nc.tensor.matmul(
                            ps[:].rearrange("p (j a) -> p j a", j=32),
                            loff[:, t, :],
                            rhs.rearrange("p a j -> p j a"),
                            start=(t == 0), stop=(t == 8))
                    om_sb = pbt.tile([54, 512], F32, tag="om_sb")
                    nc.scalar.copy(om_sb[:], ps[:])
                    for q in range(3):
                        nc.gpsimd.dma_start(
                            out=q_in[:, q, cb2 * 512:(cb2 + 1) * 512],
                            in_=om_sb[q * 18:(q + 1) * 18, :])

                qd = pbt.tile([18, 7, 1024], F32, tag="qd")
                qb = pbt.tile([18, 5, 1024], BF16, tag="qb")
                qcc = pbt.tile([18, 4, 1024], BF16, tag="qcc")
                sl = slice(ch * 1024, (ch + 1) * 1024)
                dy, dx, mm = q_in[:, 0, :], q_in[:, 1, :], q_in[:, 2, :]
                y_, x_ = qd[:, 0, :], qd[:, 1, :]
                t1, t2 = qd[:, 2, :], qd[:, 3, :]
                y0, x0 = qd[:, 4, :], qd[:, 5, :]
                idxf = qd[:, 6, :]
                wy, wx, m_ = qb[:, 0, :], qb[:, 1, :], qb[:, 2, :]
                u_, t_ = qb[:, 3, :], qb[:, 4, :]

                nc.vector.scalar_tensor_tensor(
                    y_, dy, cst[0:18, 6:7], yadd[:, sl], AL.add, AL.add)
                nc.vector.scalar_tensor_tensor(
                    x_, dx, cst[0:18, 7:8], xadd[:, sl], AL.add, AL.add)
                # y' = y - 0.5 (folded into yadd); y0 = RNE(y') = floor(y)
                # except at exact-integer y, where wy=1 keeps bilinear exact.
                nc.scalar.activation(t1, y_, AF.Identity, bias=cst[0:18, 9:10])
                nc.scalar.activation(y0, t1, AF.Identity, bias=cst[0:18, 10:11])
                nc.scalar.activation(t2, x_, AF.Identity, bias=cst[0:18, 9:10])
                nc.scalar.activation(x0, t2, AF.Identity, bias=cst[0:18, 10:11])
                nc.vector.scalar_tensor_tensor(wy, y_, 0.5, y0,
                                               AL.add, AL.subtract)
                nc.vector.scalar_tensor_tensor(wx, x_, 0.5, x0,
                                               AL.add, AL.subtract)
                nc.vector.scalar_tensor_tensor(idxf, y0, float(FC), x0,
                                               AL.mult, AL.add)
                nc.vector.tensor_scalar(idxf, idxf, float(IDXMAX), 0.0,
                                        AL.min, AL.max)
                nc.scalar.copy(
                    out=idx16[:, :, ch * 64:(ch + 1) * 64]
                        .rearrange("p q j -> p j q"),
                    in_=idxf)

                nc.scalar.activation(m_, mm, AF.Sigmoid, bias=cst[0:18, 8:9])
                nc.vector.tensor_tensor(u_, m_, wy, AL.mult)
                nc.vector.tensor_tensor(t_, m_, u_, AL.subtract)
                nc.vector.tensor_tensor(qcc[:, 3, :], u_, wx, AL.mult)
                nc.vector.tensor_tensor(qcc[:, 2, :], u_, qcc[:, 3, :],
                                        AL.subtract)
                nc.vector.tensor_tensor(qcc[:, 1, :], t_, wx, AL.mult)
                nc.vector.tensor_tensor(qcc[:, 0, :], t_, qcc[:, 1, :],
                                        AL.subtract)
                for j4 in range(4):
                    nc.sync.dma_start(out=d_cm[j4, :, sl],
                                      in_=qcc[:, j4, :])
                nc.sync.dma_start(
                    out=d_iw[:, :, ch * 64:(ch + 1) * 64],
                    in_=idx16[:, :, ch * 64:(ch + 1) * 64])

        # ================= Phase C: gather + hadamard + einsum =================
        with tc.tile_pool(name="idxp", bufs=1) as idxp, \
             tc.tile_pool(name="cbp", bufs=3) as cbp, \
             tc.tile_pool(name="vp", bufs=2) as vp, \
             tc.tile_pool(name="pp", bufs=3) as ppool, \
             tc.tile_pool(name="psum_c", bufs=1, space="PSUM") as psc, \
             tc.tile_pool(name="outp", bufs=1) as outp:
            pos = psc.tile([P, S], F32)
            fflat = fpadA[:].rearrange("p a b -> p (a b)")
            # idx layout [p, k, hf, corner, 128]: a half's 4 corner lists are
            # contiguous so one 4-corner, 8192-idx gather covers them (the
            # cost model charges max(out_free, num_elems) per gather, so
            # batch the output well past the 5360-element source scan).
            idxall = idxp.tile([P, 9, 2, 4, 128], I16)
            HS = S // 2
            for hf in range(2):
                for k in range(9):
                    for dd in range(2):
                        nc.sync.dma_start(
                            out=idxall[dd * 64:(dd + 1) * 64, k, hf, 0, :],
                            in_=d_iw[2 * k + dd, None, :, hf * 128:(hf + 1) * 128]
                                .to_broadcast([4, 16, 128]))
                for k in range(9):
                    for sl4, ofs in ((1, 1), (2, FC), (3, FC + 1)):
                        nc.vector.tensor_scalar_add(
                            idxall[:, k, hf, sl4, :],
                            idxall[:, k, hf, 0, :], ofs)
                for k in range(9):
                    v = vp.tile([P, 2 * S], F32, tag="v")
                    nc.gpsimd.ap_gather(
                        out_ap=v[:], in_ap=fflat,
                        idxs_ap=idxall[:, k, hf].rearrange("p a b -> p (a b)"),
                        channels=P, num_elems=NELEM, d=1, num_idxs=2 * S)
                    for j4 in range(4):
                        cb = cbp.tile([P, HS], BF16, tag="cb")
                        nc.sync.dma_start(
                            out=cb[:],
                            in_=d_cm[j4, 2 * k: 2 * k + 2, None,
                                     hf * HS:(hf + 1) * HS]
                                .to_broadcast([2, 64, HS]))
                        pt = ppool.tile([P, HS], BF16, tag="pt")
                        nc.vector.tensor_tensor(
                            pt[:], v[:, j4 * HS:(j4 + 1) * HS], cb[:], AL.mult)
                        for b4 in range(4):
                            nc.tensor.matmul(
                                pos[:, hf * HS + b4 * 512:
                                    hf * HS + (b4 + 1) * 512],
                                ldcn[:, k, :],
                                pt[:, b4 * 512:(b4 + 1) * 512],
                                start=(k == 0 and j4 == 0),
                                stop=(k == 8 and j4 == 3))

            # out stage: bn3 + relu, un-permute i-order -> (y, x)
            ob = outp.tile([P, S], F32)
            obp = ob[:].rearrange("p (a b x) -> p b x a", a=16, b=2)
            posp = pos[:].rearrange("p (b x a) -> p b x a", b=2, x=128)
            for oh in range(2):
                nc.scalar.activation(obp[:, oh], posp[:, oh],
                                     AF.Relu, bias=beta3, scale=inv3)
            nc.sync.dma_start(out=d_out[:],
                              in_=ob[:].rearrange("p (y x) -> p y x", y=QROWS))

    nc.compile()
    return nc


_CACHE = {}


def _prep(inputs):
    f = {k: _f(v) for k, v in inputs.items()}
    inv1 = f['g1'] / np.sqrt(f['v1'] + EPS)
    beta1 = f['b1'] - f['m1'] * inv1
    inv2 = f['g2'] / np.sqrt(f['v2'] + EPS)
    beta2 = f['b2'] - f['m2'] * inv2
    invd = f['gd'] / np.sqrt(f['vd'] + EPS)
    betad = f['bd'] - f['md'] * invd
    inv3 = f['g3'] / np.sqrt(f['v3'] + EPS)
    beta3 = f['b3'] - f['m3'] * inv3

    lhsT1 = np.transpose(f['w1'], (1, 2, 3, 0)).reshape(Ci, 9, P)
    lhsT2 = np.transpose(f['w2'], (1, 2, 3, 0)).reshape(P, 9, P)
    wd = f['wd'][:, :, 0, 0] * (invd / inv2)[:, None]
    lhsT_sc = np.ascontiguousarray(wd.T)

    # offset conv rows: quant*18 + k*2 + d  <-  orig quant*18 + d*9 + k
    perm = np.zeros(54, dtype=np.int64)
    for quant in range(3):
        for kk in range(9):
            for dd in range(2):
                perm[quant * 18 + kk * 2 + dd] = quant * 18 + dd * 9 + kk
    ow = f['off_w'][perm]
    obias = f['off_b'][perm]
    lhsT_off = np.transpose(ow, (1, 2, 3, 0)).reshape(P, 9, 54)

    wr = f['dcn_w'].reshape(Co, DG, Cg, 9)
    lhsT_dcn = np.transpose(wr, (1, 2, 3, 0)).reshape(P, 9, Co)

    cst = np.zeros((P, 12), dtype=np.float32)
    cst[:, 9], cst[:, 10] = MAGIC, -MAGIC
    cst[:, 0], cst[:, 1] = inv1, beta1
    cst[:, 2], cst[:, 3] = inv2, beta2 + betad
    cst[:, 4], cst[:, 5] = inv3, beta3 + inv3 * f['dcn_b']
    for kd in range(18):
        cst[kd, 6] = obias[0 * 18 + kd]   # dy bias
        cst[kd, 7] = obias[1 * 18 + kd]   # dx bias
        cst[kd, 8] = obias[2 * 18 + kd]   # mask bias

    # i-order position constants: i = ((y%2)*128 + x)*16 + y//2
    ii = np.arange(S)
    aa = ii % 16
    cc = ii // 16
    bb2 = cc // 128
    xx = cc % 128
    yloc = 2 * aa + bb2
    y_add = np.zeros((18, S), dtype=np.float32)
    x_add = np.zeros((18, S), dtype=np.float32)
    for kk in range(9):
        for dd in range(2):
            kd = 2 * kk + dd
            y_add[kd] = yloc + (kk // 3) + 1.5
            x_add[kd] = xx + (kk % 3) + 1.5

    return dict(
        lhsT1=_f(lhsT1), lhsT2=_bf(lhsT2), lhsT_sc=_f(lhsT_sc),
        lhsT_off=_bf(lhsT_off), lhsT_dcn=_bf(lhsT_dcn),
        consts=_f(cst), y_add=_f(y_add), x_add=_f(x_add), x=f['x'])


def kernel(**inputs):
    cfg = _prep(inputs)
    x = cfg.pop('x')
    B = x.shape[0]

    if 'nc' not in _CACHE:
        _CACHE['nc'] = build_nc()
    nc = _CACHE['nc']

    in_maps = []
    for cid in range(8):
        b, q = cid // 4, cid % 4
        h0 = 32 * q
        xp = np.zeros((Ci, XR, XC), dtype=np.float32)
        r_lo = 2 * h0 - 9
        s_lo, s_hi = max(r_lo, 0), min(2 * h0 + 72, 256)
        xp[:, s_lo - r_lo: s_hi - r_lo, 1:257] = x[b, :, s_lo:s_hi, :]
        rm1 = np.zeros((P, F1R), dtype=np.float32)
        for f1 in range(F1R):
            rm1[:, f1] = 1.0 if 0 <= h0 - 4 + f1 < H else 0.0
        rmf = np.zeros((P, FR), dtype=np.float32)
        for f2 in range(38):
            rmf[:, f2] = 1.0 if 0 <= h0 - 3 + f2 < H else 0.0
        m = dict(cfg)
        m['x_shard'] = np.ascontiguousarray(xp)
        m['rowmask1'] = rm1
        m['rowmaskF'] = rmf
        in_maps.append(m)

    res = run_bass_kernel_spmd(nc, in_maps, core_ids=list(range(8)))
    out = np.zeros((B, Co, H, W), dtype=np.float32)
    for cid in range(8):
        b, q = cid // 4, cid % 4
        out[b, :, 32 * q:32 * q + 32, :] = res.results[cid]['out']
    return out


# revision 24
# speedup vs baseline: 1.4182x; 1.0509x over previous
"""Trainium2 Bass kernel for nn_DeforConv_71605694759687 (gather-based).

ResBlock(stride2, 64->128) + DCNv2 (modulated deformable conv) + BN + ReLU.

Sharding (8 cores): (batch b = core//4, H-quarter q = core%4); each core
computes 32 output rows of out[b] end-to-end locally (halo via recompute,
no collectives).

Unlike the tent-expansion design, deformable sampling here uses real
GPSIMD gathers (ap_gather): per 3x3 tap k, the four bilinear corner
values are gathered from the padded feature map at runtime-computed
int16 indices, multiplied by per-corner coefficient maps
(mask * bilinear weights, broadcast from 18 rows to 128 partitions via
DRAM-bounce replication DMAs), and contracted on the PE with the DCN
weights (9 taps x 4 corners accumulating matmuls).

Spatial positions use the "i-order" i = ((y%2)*128 + x)*16 + y//2 so
that the ap_gather 16-partition index wrap, the offset-conv rhs view,
and the output un-permute are all regular strided APs.
"""

import numpy as np
import ml_dtypes
from contextlib import ExitStack

import concourse.bass as bass
import concourse.tile as tile
from concourse import mybir, bacc
from concourse.bass_utils import run_bass_kernel_spmd

F32 = mybir.dt.float32
F32R = mybir.dt.float32r
BF16 = mybir.dt.bfloat16
I16 = mybir.dt.int16
AL = mybir.AluOpType
AF = mybir.ActivationFunctionType

P = 128
EPS = 1e-5
Ci, Co, DG, Cg = 64, 128, 2, 64
H, W = 128, 128          # output spatial (after stride-2)
QROWS = 32               # output rows per core
FR = 40                  # F_pad rows: h0-3 .. h0+34 (+2 zero guard rows)
FC = 134                 # F_pad cols: x in [-3, 130]
NELEM = FR * FC          # 5360 gather elements per partition
F1R, F1C = 40, 130       # feat1: rows h0-4..h0+35, cols [-1,128]
XR, XC = 81, 258         # x_pad: rows 2*h0-9..2*h0+71, cols [-1,256]
S = 4096                 # spatial positions per core (32*128)
MAGIC = 12582912.0       # 1.5 * 2^23, fp32 RNE rounding trick
IDXMAX = 37 * FC + 132   # max legal idx00


def _bf(x):
    return np.ascontiguousarray(np.asarray(x).astype(ml_dtypes.bfloat16))


def _f(x):
    return np.ascontiguousarray(np.asarray(x, dtype=np.float32))


def build_nc():
    nc = bacc.Bacc(None)

    d_x = nc.dram_tensor("x_shard", [Ci, XR, XC], F32R, kind="ExternalInput")
    d_l1 = nc.dram_tensor("lhsT1", [Ci, 9, P], F32R, kind="ExternalInput")
    d_l2 = nc.dram_tensor("lhsT2", [P, 9, P], BF16, kind="ExternalInput")
    d_lsc = nc.dram_tensor("lhsT_sc", [Ci, P], F32R, kind="ExternalInput")
    d_loff = nc.dram_tensor("lhsT_off", [P, 9, 54], BF16, kind="ExternalInput")
    d_ldcn = nc.dram_tensor("lhsT_dcn", [P, 9, P], BF16, kind="ExternalInput")
    d_cst = nc.dram_tensor("consts", [P, 12], F32, kind="ExternalInput")
    d_yadd = nc.dram_tensor("y_add", [18, S], BF16, kind="ExternalInput")
    d_xadd = nc.dram_tensor("x_add", [18, S], BF16, kind="ExternalInput")
    d_rm1 = nc.dram_tensor("rowmask1", [P, F1R], F32, kind="ExternalInput")
    d_rmf = nc.dram_tensor("rowmaskF", [P, FR], F32, kind="ExternalInput")
    d_out = nc.dram_tensor("out", [P, QROWS, W], F32, kind="ExternalOutput")

    d_cm = nc.dram_tensor("cmaps", [4, 18, S], BF16, kind="Internal")
    d_iw = nc.dram_tensor("idxw", [18, 16, 256], I16, kind="Internal")

    with tile.TileContext(nc) as tc, ExitStack() as ctx:
        singles = ctx.enter_context(tc.tile_pool(name="singles", bufs=1))

        fpadA = singles.tile([P, FR, FC], F32)     # gather source, col c <-> x-3
        ldcn = singles.tile([P, 9, P], BF16)
        cst = singles.tile([P, 12], F32)
        nc.sync.dma_start(out=ldcn[:], in_=d_ldcn[:])
        nc.sync.dma_start(out=cst[:], in_=d_cst[:])

        inv1, beta1 = cst[:, 0:1], cst[:, 1:2]
        inv2, beta2 = cst[:, 2:3], cst[:, 3:4]
        inv3, beta3 = cst[:, 4:5], cst[:, 5:6]

        nc.vector.memset(fpadA[:, :, 0:3], 0.0)
        nc.vector.memset(fpadA[:, :, FC - 3:FC], 0.0)
        nc.vector.memset(fpadA[:, 38:40, :], 0.0)

        # ================= Phase A: ResBlock =================
        with tc.tile_pool(name="ph_a", bufs=1) as pa, \
             tc.tile_pool(name="psum_a", bufs=2, space="PSUM") as psa:
            x_pad = pa.tile([Ci, XR, XC], F32R)
            feat1 = pa.tile([P, F1R, F1C], BF16)
            l1 = pa.tile([Ci, 9, P], F32R)
            l2 = pa.tile([P, 9, P], BF16)
            lsc = pa.tile([Ci, P], F32R)
            rm1 = pa.tile([P, F1R], F32)
            rmf = pa.tile([P, FR], F32)
            for t, dref in ((l1, d_l1), (l2, d_l2),
                            (lsc, d_lsc), (rm1, d_rm1), (rmf, d_rmf)):
                nc.sync.dma_start(out=t[:], in_=dref[:])
            for i in range(8):
                r0, r1 = (XR * i) // 8, (XR * (i + 1)) // 8
                nc.sync.dma_start(out=x_pad[:, r0:r1, :],
                                  in_=d_x[:, r0:r1, :])

            nc.vector.memset(feat1[:, :, 0:1], 0.0)
            nc.vector.memset(feat1[:, :, F1C - 1:F1C], 0.0)

            # conv1 3x3 s2 + bn1 + relu -> feat1 (bf16), interleaved with
            # conv2 3x3 s1 (+ folded shortcut) so the PE never waits long on
            # the feat1 row masks (emitted as 10-row slices).
            def conv1_block(cki):
                r0 = cki * 4
                ps = psa.tile([P, 4, W], F32)
                for t in range(9):
                    ty, tx = t // 3, t % 3
                    rhs = x_pad[:, 2 * r0 + ty: 2 * r0 + ty + 7: 2,
                                tx: tx + 2 * W - 1: 2]
                    nc.tensor.matmul(ps[:], l1[:, t, :], rhs,
                                     start=(t == 0), stop=(t == 8))
                nc.scalar.activation(feat1[:, r0:r0 + 4, 1:1 + W], ps[:],
                                     AF.Relu, bias=beta1, scale=inv1)

            def mask_slice(ms):
                r0, r1 = ms * 10, (ms + 1) * 10
                nc.vector.tensor_tensor(
                    feat1[:, r0:r1], feat1[:, r0:r1],
                    rm1[:, r0:r1, None].to_broadcast((P, 10, F1C)), AL.mult)

            def conv2_block(cki):
                r0 = cki * 4
                nrow = min(4, 38 - r0)
                ps = psa.tile([P, 4, W], F32, tag="ps2")
                for t in range(9):
                    ty, tx = t // 3, t % 3
                    rhs = feat1[:, r0 + ty: r0 + ty + nrow, tx: tx + W]
                    nc.tensor.matmul(ps[:, :nrow], l2[:, t, :], rhs,
                                     start=(t == 0), stop=False)
                rhs_sc = x_pad[:, 2 * r0 + 3: 2 * r0 + 2 + 2 * nrow: 2,
                               1: 2 * W: 2]
                nc.tensor.matmul(ps[:, :nrow], lsc[:], rhs_sc,
                                 start=False, stop=True)
                nc.scalar.activation(fpadA[:, r0:r0 + nrow, 3:3 + W],
                                     ps[:, :nrow], AF.Relu,
                                     bias=beta2, scale=inv2)

            done1, done2, donem = 0, 0, 0
            for step in range(40):
                if done1 < 10:
                    conv1_block(done1)
                    done1 += 1
                while donem < 4 and done1 * 4 >= (donem + 1) * 10 + 2:
                    mask_slice(donem)
                    donem += 1
                while (done2 < 10 and donem * 10 >= min(done2 * 4 + 6, 38)
                       and done1 * 4 >= done2 * 4 + 6):
                    conv2_block(done2)
                    done2 += 1
                    if done2 % 2 == 0:
                        break
                if done1 == 10 and done2 == 10:
                    break
            nc.vector.tensor_tensor(
                fpadA[:, 0:38], fpadA[:, 0:38],
                rmf[:, 0:38, None].to_broadcast((P, 38, FC)), AL.mult)

        # ================= Phase B: offsets -> idx + coeff maps =================
        # All per-(k,d) quantities live on partitions 0..17 with the quantity
        # index in the free dim (engines cannot cross partition bases).
        # Processed in 4 chunks of 1024 spatial positions (2 om blocks each).
        idxp = ctx.enter_context(tc.tile_pool(name="idxp", bufs=1))
        idxall = idxp.tile([P, 9, 2, 4, 128], I16)

        def idx_prep(hf):
            # replicate the wrapped idx lists to all four 16-partition groups
            # per deform group, then derive the +1/+W/+W+1 corner variants.
            for k in range(9):
                for dd in range(2):
                    nc.scalar.dma_start(
                        out=idxall[dd * 64:(dd + 1) * 64, k, hf, 0, :],
                        in_=d_iw[2 * k + dd, None, :, hf * 128:(hf + 1) * 128]
                            .to_broadcast([4, 16, 128]))
                for sl4, ofs in ((1, 1), (2, FC), (3, FC + 1)):
                    nc.vector.tensor_scalar_add(
                        idxall[:, k, hf, sl4, :],
                        idxall[:, k, hf, 0, :], ofs)

        with tc.tile_pool(name="ph_b", bufs=1) as pb, \
             tc.tile_pool(name="ph_b_q", bufs=2) as pbq, \
             tc.tile_pool(name="ph_b_tmp", bufs=2) as pbt, \
             tc.tile_pool(name="psum_b", bufs=2, space="PSUM") as psb:
            fpadB = pb.tile([P, FR, FC], BF16)
            loff = pb.tile([P, 9, 54], BF16)
            yadd = pb.tile([18, S], BF16)
            xadd = pb.tile([18, S], BF16)
            idx16 = pb.tile([18, 16, 256], I16)   # wrapped (p, j) layout
            nc.sync.dma_start(out=loff[:], in_=d_loff[:])
            nc.sync.dma_start(out=yadd[:], in_=d_yadd[:])
            nc.sync.dma_start(out=xadd[:], in_=d_xadd[:])
            nc.vector.tensor_copy(out=fpadB[:], in_=fpadA[:])

            for ch in range(4):
                # q_in rows 0..17, free: [quant, 1024]
                q_in = pbq.tile([18, 3, 1024], F32, tag="q_in")
                for cb2 in range(2):
                    cki = 2 * ch + cb2
                    # offset conv: out channels (quant*18 + 2k+d); i-order.
                    # block cki covers i in [512cki, 512cki+512):
                    # y = 2a + cki//4, x = 32*(cki%4) + j';
                    # rhs rows y+2+ty, cols x+2+tx, (j' outer, a inner)
                    b2, xq = cki // 4, 32 * (cki % 4)
                    ps = psb.tile([54, 512], F32)
                    for t in range(9):
                        ty, tx = t // 3, t % 3
                        rhs = fpadB[:, b2 + 2 + ty: b2 + 2 + ty + 32: 2,
                                    xq + 2 + tx: xq + 2 + tx + 32]
                        nc.tensor.matmul(
                            ps[:].rearrange("p (j a) -> p j a", j=32),
                            loff[:, t, :],
                            rhs.rearrange("p a j -> p j a"),
                            start=(t == 0), stop=(t == 8))
                    om_sb = pbt.tile([54, 512], F32, tag="om_sb")
                    nc.scalar.copy(om_sb[:], ps[:])
                    for q in range(3):
                        nc.gpsimd.dma_start(
                            out=q_in[:, q, cb2 * 512:(cb2 + 1) * 512],
                            in_=om_sb[q * 18:(q + 1) * 18, :])

                qd = pbt.tile([18, 6, 1024], F32, tag="qd")
                qb = pbt.tile([18, 5, 1024], BF16, tag="qb")
                qcc = pbt.tile([18, 4, 1024], BF16, tag="qcc")
                sl = slice(ch * 1024, (ch + 1) * 1024)
                dy, dx, mm = q_in[:, 0, :], q_in[:, 1, :], q_in[:, 2, :]
                y_, x_ = qd[:, 0, :], qd[:, 1, :]
                t1, t2 = qd[:, 2, :], qd[:, 2, :]
                y0, x0 = qd[:, 3, :], qd[:, 4, :]
                idxf = qd[:, 5, :]
                wy, wx, m_ = qb[:, 0, :], qb[:, 1, :], qb[:, 2, :]
                u_, t_ = qb[:, 3, :], qb[:, 4, :]

                nc.vector.scalar_tensor_tensor(
                    y_, dy, cst[0:18, 6:7], yadd[:, sl], AL.add, AL.add)
                nc.vector.scalar_tensor_tensor(
                    x_, dx, cst[0:18, 7:8], xadd[:, sl], AL.add, AL.add)
                # y' = y - 0.5 (folded into yadd); y0 = RNE(y') = floor(y)
                # except at exact-integer y, where wy=1 keeps bilinear exact.
                nc.scalar.activation(t1, y_, AF.Identity, bias=cst[0:18, 9:10])
                nc.scalar.activation(y0, t1, AF.Identity, bias=cst[0:18, 10:11])
                nc.scalar.activation(t2, x_, AF.Identity, bias=cst[0:18, 9:10])
                nc.scalar.activation(x0, t2, AF.Identity, bias=cst[0:18, 10:11])
                nc.vector.scalar_tensor_tensor(wy, y_, 0.5, y0,
                                               AL.add, AL.subtract)
                nc.vector.scalar_tensor_tensor(wx, x_, 0.5, x0,
                                               AL.add, AL.subtract)
                nc.vector.scalar_tensor_tensor(idxf, y0, float(FC), x0,
                                               AL.mult, AL.add)
                nc.vector.tensor_scalar(idxf, idxf, float(IDXMAX), 0.0,
                                        AL.min, AL.max)
                nc.scalar.copy(
                    out=idx16[:, :, ch * 64:(ch + 1) * 64]
                        .rearrange("p q j -> p j q"),
                    in_=idxf)

                nc.scalar.activation(m_, mm, AF.Sigmoid, bias=cst[0:18, 8:9])
                nc.vector.tensor_tensor(u_, m_, wy, AL.mult)
                nc.vector.tensor_tensor(t_, m_, u_, AL.subtract)
                nc.vector.tensor_tensor(qcc[:, 3, :], u_, wx, AL.mult)
                nc.vector.tensor_tensor(qcc[:, 2, :], u_, qcc[:, 3, :],
                                        AL.subtract)
                nc.vector.tensor_tensor(qcc[:, 1, :], t_, wx, AL.mult)
                nc.vector.tensor_tensor(qcc[:, 0, :], t_, qcc[:, 1, :],
                                        AL.subtract)
                for j4 in range(4):
                    nc.scalar.dma_start(out=d_cm[j4, :, sl],
                                        in_=qcc[:, j4, :])
                nc.scalar.dma_start(
                    out=d_iw[:, :, ch * 64:(ch + 1) * 64],
                    in_=idx16[:, :, ch * 64:(ch + 1) * 64])


        # ================= Phase C: gather + hadamard + einsum =================
        with tc.tile_pool(name="cbp", bufs=3) as cbp, \
             tc.tile_pool(name="vp", bufs=2) as vp, \
             tc.tile_pool(name="pp", bufs=3) as ppool, \
             tc.tile_pool(name="psum_c", bufs=1, space="PSUM") as psc, \
             tc.tile_pool(name="outp", bufs=1) as outp:
            pos = psc.tile([P, S], F32)
            fflat = fpadA[:].rearrange("p a b -> p (a b)")
            # idx layout [p, k, hf, corner, 128]: a half's 4 corner lists are
            # contiguous so one 4-corner, 8192-idx gather covers them (the
            # cost model charges max(out_free, num_elems) per gather, so
            # batch the output well past the 5360-element source scan).
            HS = S // 2
            idx_prep(0)
            idx_prep(1)
            for hf in range(2):
                for k in range(9):
                    v = vp.tile([P, 2 * S], F32, tag="v")
                    nc.gpsimd.ap_gather(
                        out_ap=v[:], in_ap=fflat,
                        idxs_ap=idxall[:, k, hf].rearrange("p a b -> p (a b)"),
                        channels=P, num_elems=NELEM, d=1, num_idxs=2 * S)
                    for j4 in range(4):
                        cb = cbp.tile([P, HS], BF16, tag="cb")
                        nc.sync.dma_start(
                            out=cb[:],
                            in_=d_cm[j4, 2 * k: 2 * k + 2, None,
                                     hf * HS:(hf + 1) * HS]
                                .to_broadcast([2, 64, HS]))
                        pt = ppool.tile([P, HS], BF16, tag="pt")
                        nc.vector.tensor_tensor(
                            pt[:], v[:, j4 * HS:(j4 + 1) * HS], cb[:], AL.mult)
                        for b4 in range(4):
                            nc.tensor.matmul(
                                pos[:, hf * HS + b4 * 512:
                                    hf * HS + (b4 + 1) * 512],
                                ldcn[:, k, :],
                                pt[:, b4 * 512:(b4 + 1) * 512],
                                start=(k == 0 and j4 == 0),
                                stop=(k == 8 and j4 == 3))

            # out stage: bn3 + relu, un-permute i-order -> (y, x)
            ob = outp.tile([P, S], F32)
            obp = ob[:].rearrange("p (a b x) -> p b x a", a=16, b=2)
            posp = pos[:].rearrange("p (b x a) -> p b x a", b=2, x=128)
            ob3 = ob[:].rearrange("p (y x) -> p y x", y=QROWS)
            for oh in range(2):
                nc.scalar.activation(obp[:, oh], posp[:, oh],
                                     AF.Relu, bias=beta3, scale=inv3)
                nc.sync.dma_start(out=d_out[:, oh::2, :],
                                  in_=ob3[:, oh::2, :])

    nc.compile()
    return nc


_CACHE = {}


def _prep(inputs):
    f = {k: _f(v) for k, v in inputs.items()}
    inv1 = f['g1'] / np.sqrt(f['v1'] + EPS)
    beta1 = f['b1'] - f['m1'] * inv1
    inv2 = f['g2'] / np.sqrt(f['v2'] + EPS)
    beta2 = f['b2'] - f['m2'] * inv2
    invd = f['gd'] / np.sqrt(f['vd'] + EPS)
    betad = f['bd'] - f['md'] * invd
    inv3 = f['g3'] / np.sqrt(f['v3'] + EPS)
    beta3 = f['b3'] - f['m3'] * inv3

    lhsT1 = np.transpose(f['w1'], (1, 2, 3, 0)).reshape(Ci, 9, P)
    lhsT2 = np.transpose(f['w2'], (1, 2, 3, 0)).reshape(P, 9, P)
    wd = f['wd'][:, :, 0, 0] * (invd / inv2)[:, None]
    lhsT_sc = np.ascontiguousarray(wd.T)

    # offset conv rows: quant*18 + k*2 + d  <-  orig quant*18 + d*9 + k
    perm = np.zeros(54, dtype=np.int64)
    for quant in range(3):
        for kk in range(9):
            for dd in range(2):
                perm[quant * 18 + kk * 2 + dd] = quant * 18 + dd * 9 + kk
    ow = f['off_w'][perm]
    obias = f['off_b'][perm]
    lhsT_off = np.transpose(ow, (1, 2, 3, 0)).reshape(P, 9, 54)

    wr = f['dcn_w'].reshape(Co, DG, Cg, 9)
    lhsT_dcn = np.transpose(wr, (1, 2, 3, 0)).reshape(P, 9, Co)

    cst = np.zeros((P, 12), dtype=np.float32)
    cst[:, 9], cst[:, 10] = MAGIC, -MAGIC
    cst[:, 0], cst[:, 1] = inv1, beta1
    cst[:, 2], cst[:, 3] = inv2, beta2 + betad
    cst[:, 4], cst[:, 5] = inv3, beta3 + inv3 * f['dcn_b']
    for kd in range(18):
        cst[kd, 6] = obias[0 * 18 + kd] + 1.5   # dy bias + pad - 0.5
        cst[kd, 7] = obias[1 * 18 + kd] + 1.5   # dx bias + pad - 0.5
        cst[kd, 8] = obias[2 * 18 + kd]   # mask bias

    # i-order position constants: i = ((y%2)*128 + x)*16 + y//2
    ii = np.arange(S)
    aa = ii % 16
    cc = ii // 16
    bb2 = cc // 128
    xx = cc % 128
    yloc = 2 * aa + bb2
    y_add = np.zeros((18, S), dtype=np.float32)
    x_add = np.zeros((18, S), dtype=np.float32)
    for kk in range(9):
        for dd in range(2):
            kd = 2 * kk + dd
            y_add[kd] = yloc + (kk // 3)
            x_add[kd] = xx + (kk % 3)

    return dict(
        lhsT1=_f(lhsT1), lhsT2=_bf(lhsT2), lhsT_sc=_f(lhsT_sc),
        lhsT_off=_bf(lhsT_off), lhsT_dcn=_bf(lhsT_dcn),
        consts=_f(cst), y_add=_bf(y_add), x_add=_bf(x_add), x=f['x'])


def kernel(**inputs):
    cfg = _prep(inputs)
    x = cfg.pop('x')
    B = x.shape[0]

    if 'nc' not in _CACHE:
        _CACHE['nc'] = build_nc()
    nc = _CACHE['nc']

    in_maps = []
    for cid in range(8):
        b, q = cid // 4, cid % 4
        h0 = 32 * q
        xp = np.zeros((Ci, XR, XC), dtype=np.float32)
        r_lo = 2 * h0 - 9
        s_lo, s_hi = max(r_lo, 0), min(2 * h0 + 72, 256)
        xp[:, s_lo - r_lo: s_hi - r_lo, 1:257] = x[b, :, s_lo:s_hi, :]
        rm1 = np.zeros((P, F1R), dtype=np.float32)
        for f1 in range(F1R):
            rm1[:, f1] = 1.0 if 0 <= h0 - 4 + f1 < H else 0.0
        rmf = np.zeros((P, FR), dtype=np.float32)
        for f2 in range(38):
            rmf[:, f2] = 1.0 if 0 <= h0 - 3 + f2 < H else 0.0
        m = dict(cfg)
        m['x_shard'] = np.ascontiguousarray(xp)
        m['rowmask1'] = rm1
        m['rowmaskF'] = rmf
        in_maps.append(m)

    out = np.zeros((B, Co, H, W), dtype=np.float32)
    for attempt in range(2):
        res = run_bass_kernel_spmd(nc, in_maps, core_ids=list(range(8)))
        for cid in range(8):
            b, q = cid // 4, cid % 4
            out[b, :, 32 * q:32 * q + 32, :] = res.results[cid]['out']
        # a previously-wedged NeuronCore can surface one garbage run;
        # a clean retry recovers it.
        if np.isfinite(out).all():
            break
    return out


# revision 25
# speedup vs baseline: 1.4434x; 1.0178x over previous
"""Trainium2 Bass kernel for nn_DeforConv_71605694759687 (gather-based).

ResBlock(stride2, 64->128) + DCNv2 (modulated deformable conv) + BN + ReLU.

Sharding (8 cores): (batch b = core//4, H-quarter q = core%4); each core
computes 32 output rows of out[b] end-to-end locally (halo via recompute,
no collectives).

Unlike the tent-expansion design, deformable sampling here uses real
GPSIMD gathers (ap_gather): per 3x3 tap k, the four bilinear corner
values are gathered from the padded feature map at runtime-computed
int16 indices, multiplied by per-corner coefficient maps
(mask * bilinear weights, broadcast from 18 rows to 128 partitions via
DRAM-bounce replication DMAs), and contracted on the PE with the DCN
weights (9 taps x 4 corners accumulating matmuls).

Spatial positions use the "i-order" i = ((y%2)*128 + x)*16 + y//2 so
that the ap_gather 16-partition index wrap, the offset-conv rhs view,
and the output un-permute are all regular strided APs.
"""

import numpy as np
import ml_dtypes
from contextlib import ExitStack

import concourse.bass as bass
import concourse.tile as tile
from concourse import mybir, bacc
from concourse.bass_utils import run_bass_kernel_spmd

F32 = mybir.dt.float32
F32R = mybir.dt.float32r
BF16 = mybir.dt.bfloat16
I16 = mybir.dt.int16
AL = mybir.AluOpType
AF = mybir.ActivationFunctionType

P = 128
EPS = 1e-5
Ci, Co, DG, Cg = 64, 128, 2, 64
H, W = 128, 128          # output spatial (after stride-2)
QROWS = 32               # output rows per core
FR = 40                  # F_pad rows: h0-3 .. h0+34 (+2 zero guard rows)
FC = 134                 # F_pad cols: x in [-3, 130]
NELEM = FR * FC          # 5360 gather elements per partition
F1R, F1C = 40, 130       # feat1: rows h0-4..h0+35, cols [-1,128]
XR, XC = 81, 258         # x_pad: rows 2*h0-9..2*h0+71, cols [-1,256]
S = 4096                 # spatial positions per core (32*128)
MAGIC = 12582912.0       # 1.5 * 2^23, fp32 RNE rounding trick
IDXMAX = 37 * FC + 132   # max legal idx00


def _bf(x):
    return np.ascontiguousarray(np.asarray(x).astype(ml_dtypes.bfloat16))


def _f(x):
    return np.ascontiguousarray(np.asarray(x, dtype=np.float32))


def build_nc():
    nc = bacc.Bacc(None)

    d_x = nc.dram_tensor("x_shard", [Ci, XR, XC], F32R, kind="ExternalInput")
    d_l1 = nc.dram_tensor("lhsT1", [Ci, 9, P], F32R, kind="ExternalInput")
    d_l2 = nc.dram_tensor("lhsT2", [P, 9, P], BF16, kind="ExternalInput")
    d_lsc = nc.dram_tensor("lhsT_sc", [Ci, P], F32R, kind="ExternalInput")
    d_loff = nc.dram_tensor("lhsT_off", [P, 9, 54], BF16, kind="ExternalInput")
    d_ldcn = nc.dram_tensor("lhsT_dcn", [P, 9, P], BF16, kind="ExternalInput")
    d_cst = nc.dram_tensor("consts", [P, 12], F32, kind="ExternalInput")
    d_yadd = nc.dram_tensor("y_add", [18, S], BF16, kind="ExternalInput")
    d_xadd = nc.dram_tensor("x_add", [18, S], BF16, kind="ExternalInput")
    d_rm1 = nc.dram_tensor("rowmask1", [P, F1R], F32, kind="ExternalInput")
    d_rmf = nc.dram_tensor("rowmaskF", [P, FR], F32, kind="ExternalInput")
    d_out = nc.dram_tensor("out", [P, QROWS, W], F32, kind="ExternalOutput")

    d_cm = nc.dram_tensor("cmaps", [4, 18, S], BF16, kind="Internal")
    d_iw = nc.dram_tensor("idxw", [18, 16, 256], I16, kind="Internal")

    with tile.TileContext(nc) as tc, ExitStack() as ctx:
        singles = ctx.enter_context(tc.tile_pool(name="singles", bufs=1))

        fpadA = singles.tile([P, FR, FC], F32)     # gather source, col c <-> x-3
        ldcn = singles.tile([P, 9, P], BF16)
        cst = singles.tile([P, 12], F32)
        nc.sync.dma_start(out=ldcn[:], in_=d_ldcn[:])
        nc.sync.dma_start(out=cst[:], in_=d_cst[:])

        inv1, beta1 = cst[:, 0:1], cst[:, 1:2]
        inv2, beta2 = cst[:, 2:3], cst[:, 3:4]
        inv3, beta3 = cst[:, 4:5], cst[:, 5:6]

        nc.vector.memset(fpadA[:, :, 0:3], 0.0)
        nc.vector.memset(fpadA[:, :, FC - 3:FC], 0.0)
        nc.vector.memset(fpadA[:, 38:40, :], 0.0)

        # ================= Phase A: ResBlock =================
        with tc.tile_pool(name="ph_a", bufs=1) as pa, \
             tc.tile_pool(name="psum_a", bufs=2, space="PSUM") as psa:
            x_pad = pa.tile([Ci, XR, XC], F32R)
            feat1 = pa.tile([P, F1R, F1C], BF16)
            l1 = pa.tile([Ci, 9, P], F32R)
            l2 = pa.tile([P, 9, P], BF16)
            lsc = pa.tile([Ci, P], F32R)
            rm1 = pa.tile([P, F1R], F32)
            rmf = pa.tile([P, FR], F32)
            for t, dref in ((l1, d_l1), (l2, d_l2),
                            (lsc, d_lsc), (rm1, d_rm1), (rmf, d_rmf)):
                nc.sync.dma_start(out=t[:], in_=dref[:])
            for i in range(8):
                r0, r1 = (XR * i) // 8, (XR * (i + 1)) // 8
                nc.sync.dma_start(out=x_pad[:, r0:r1, :],
                                  in_=d_x[:, r0:r1, :])

            nc.vector.memset(feat1[:, :, 0:1], 0.0)
            nc.vector.memset(feat1[:, :, F1C - 1:F1C], 0.0)

            # PE warm-up in the x_pad DMA shadow: ~5us of junk matmuls ramps
            # the p-state so conv1 runs at full clock from its first block.
            warm = pa.tile([Ci, 512], BF16)
            wps = psa.tile([P, 512], F32, tag="warm")
            nc.vector.memset(warm[:], 0.0)
            for wi in range(24):
                nc.tensor.matmul(wps[:], warm[:, 0:128], warm[:],
                                 start=(wi == 0), stop=(wi == 23))

            # conv1 3x3 s2 + bn1 + relu -> feat1 (bf16), interleaved with
            # conv2 3x3 s1 (+ folded shortcut) so the PE never waits long on
            # the feat1 row masks (emitted as 10-row slices).
            def conv1_block(cki):
                r0 = cki * 4
                ps = psa.tile([P, 4, W], F32)
                for t in range(9):
                    ty, tx = t // 3, t % 3
                    rhs = x_pad[:, 2 * r0 + ty: 2 * r0 + ty + 7: 2,
                                tx: tx + 2 * W - 1: 2]
                    nc.tensor.matmul(ps[:], l1[:, t, :], rhs,
                                     start=(t == 0), stop=(t == 8))
                nc.scalar.activation(feat1[:, r0:r0 + 4, 1:1 + W], ps[:],
                                     AF.Relu, bias=beta1, scale=inv1)

            def mask_slice(ms):
                r0, r1 = ms * 10, (ms + 1) * 10
                nc.vector.tensor_tensor(
                    feat1[:, r0:r1], feat1[:, r0:r1],
                    rm1[:, r0:r1, None].to_broadcast((P, 10, F1C)), AL.mult)

            def conv2_block(cki):
                r0 = cki * 4
                nrow = min(4, 38 - r0)
                ps = psa.tile([P, 4, W], F32, tag="ps2")
                for t in range(9):
                    ty, tx = t // 3, t % 3
                    rhs = feat1[:, r0 + ty: r0 + ty + nrow, tx: tx + W]
                    nc.tensor.matmul(ps[:, :nrow], l2[:, t, :], rhs,
                                     start=(t == 0), stop=False)
                rhs_sc = x_pad[:, 2 * r0 + 3: 2 * r0 + 2 + 2 * nrow: 2,
                               1: 2 * W: 2]
                nc.tensor.matmul(ps[:, :nrow], lsc[:], rhs_sc,
                                 start=False, stop=True)
                nc.scalar.activation(fpadA[:, r0:r0 + nrow, 3:3 + W],
                                     ps[:, :nrow], AF.Relu,
                                     bias=beta2, scale=inv2)

            done1, done2, donem = 0, 0, 0
            for step in range(40):
                if done1 < 10:
                    conv1_block(done1)
                    done1 += 1
                while donem < 4 and done1 * 4 >= (donem + 1) * 10 + 2:
                    mask_slice(donem)
                    donem += 1
                while (done2 < 10 and donem * 10 >= min(done2 * 4 + 6, 38)
                       and done1 * 4 >= done2 * 4 + 6):
                    conv2_block(done2)
                    done2 += 1
                    if done2 % 2 == 0:
                        break
                if done1 == 10 and done2 == 10:
                    break
            nc.vector.tensor_tensor(
                fpadA[:, 0:38], fpadA[:, 0:38],
                rmf[:, 0:38, None].to_broadcast((P, 38, FC)), AL.mult)

        # ================= Phase B: offsets -> idx + coeff maps =================
        # All per-(k,d) quantities live on partitions 0..17 with the quantity
        # index in the free dim (engines cannot cross partition bases).
        # Processed in 4 chunks of 1024 spatial positions (2 om blocks each).
        idxp = ctx.enter_context(tc.tile_pool(name="idxp", bufs=1))
        idxall = idxp.tile([P, 9, 2, 4, 128], I16)

        def idx_prep(hf):
            # replicate the wrapped idx lists to all four 16-partition groups
            # per deform group, then derive the +1/+W/+W+1 corner variants.
            for k in range(9):
                for dd in range(2):
                    nc.scalar.dma_start(
                        out=idxall[dd * 64:(dd + 1) * 64, k, hf, 0, :],
                        in_=d_iw[2 * k + dd, None, :, hf * 128:(hf + 1) * 128]
                            .to_broadcast([4, 16, 128]))
                for sl4, ofs in ((1, 1), (2, FC), (3, FC + 1)):
                    nc.vector.tensor_scalar_add(
                        idxall[:, k, hf, sl4, :],
                        idxall[:, k, hf, 0, :], ofs)

        with tc.tile_pool(name="ph_b", bufs=1) as pb, \
             tc.tile_pool(name="ph_b_q", bufs=2) as pbq, \
             tc.tile_pool(name="ph_b_tmp", bufs=2) as pbt, \
             tc.tile_pool(name="psum_b", bufs=2, space="PSUM") as psb:
            fpadB = pb.tile([P, FR, FC], BF16)
            loff = pb.tile([P, 9, 54], BF16)
            yadd = pb.tile([18, S], BF16)
            xadd = pb.tile([18, S], BF16)
            idx16 = pb.tile([18, 16, 256], I16)   # wrapped (p, j) layout
            nc.sync.dma_start(out=loff[:], in_=d_loff[:])
            nc.sync.dma_start(out=yadd[:], in_=d_yadd[:])
            nc.sync.dma_start(out=xadd[:], in_=d_xadd[:])
            nc.vector.tensor_copy(out=fpadB[:], in_=fpadA[:])

            for ch in range(4):
                # q_in rows 0..17, free: [quant, 1024]
                q_in = pbq.tile([18, 3, 1024], F32, tag="q_in")
                for cb2 in range(2):
                    cki = 2 * ch + cb2
                    # offset conv: out channels (quant*18 + 2k+d); i-order.
                    # block cki covers i in [512cki, 512cki+512):
                    # y = 2a + cki//4, x = 32*(cki%4) + j';
                    # rhs rows y+2+ty, cols x+2+tx, (j' outer, a inner)
                    b2, xq = cki // 4, 32 * (cki % 4)
                    ps = psb.tile([54, 512], F32)
                    for t in range(9):
                        ty, tx = t // 3, t % 3
                        rhs = fpadB[:, b2 + 2 + ty: b2 + 2 + ty + 32: 2,
                                    xq + 2 + tx: xq + 2 + tx + 32]
                        nc.tensor.matmul(
                            ps[:].rearrange("p (j a) -> p j a", j=32),
                            loff[:, t, :],
                            rhs.rearrange("p a j -> p j a"),
                            start=(t == 0), stop=(t == 8))
                    om_sb = pbt.tile([54, 512], F32, tag="om_sb")
                    nc.scalar.copy(om_sb[:], ps[:])
                    for q in range(3):
                        nc.gpsimd.dma_start(
                            out=q_in[:, q, cb2 * 512:(cb2 + 1) * 512],
                            in_=om_sb[q * 18:(q + 1) * 18, :])

                qd = pbt.tile([18, 6, 1024], F32, tag="qd")
                qb = pbt.tile([18, 5, 1024], BF16, tag="qb")
                qcc = pbt.tile([18, 4, 1024], BF16, tag="qcc")
                sl = slice(ch * 1024, (ch + 1) * 1024)
                dy, dx, mm = q_in[:, 0, :], q_in[:, 1, :], q_in[:, 2, :]
                y_, x_ = qd[:, 0, :], qd[:, 1, :]
                t1, t2 = qd[:, 2, :], qd[:, 2, :]
                y0, x0 = qd[:, 3, :], qd[:, 4, :]
                idxf = qd[:, 5, :]
                wy, wx, m_ = qb[:, 0, :], qb[:, 1, :], qb[:, 2, :]
                u_, t_ = qb[:, 3, :], qb[:, 4, :]

                nc.vector.scalar_tensor_tensor(
                    y_, dy, cst[0:18, 6:7], yadd[:, sl], AL.add, AL.add)
                nc.vector.scalar_tensor_tensor(
                    x_, dx, cst[0:18, 7:8], xadd[:, sl], AL.add, AL.add)
                # y' = y - 0.5 (folded into yadd); y0 = RNE(y') = floor(y)
                # except at exact-integer y, where wy=1 keeps bilinear exact.
                nc.scalar.activation(t1, y_, AF.Identity, bias=cst[0:18, 9:10])
                nc.scalar.activation(y0, t1, AF.Identity, bias=cst[0:18, 10:11])
                nc.scalar.activation(t2, x_, AF.Identity, bias=cst[0:18, 9:10])
                nc.scalar.activation(x0, t2, AF.Identity, bias=cst[0:18, 10:11])
                nc.vector.scalar_tensor_tensor(wy, y_, 0.5, y0,
                                               AL.add, AL.subtract)
                nc.vector.scalar_tensor_tensor(wx, x_, 0.5, x0,
                                               AL.add, AL.subtract)
                nc.vector.scalar_tensor_tensor(idxf, y0, float(FC), x0,
                                               AL.mult, AL.add)
                nc.vector.tensor_scalar(idxf, idxf, float(IDXMAX), 0.0,
                                        AL.min, AL.max)
                nc.scalar.copy(
                    out=idx16[:, :, ch * 64:(ch + 1) * 64]
                        .rearrange("p q j -> p j q"),
                    in_=idxf)

                nc.scalar.activation(m_, mm, AF.Sigmoid, bias=cst[0:18, 8:9])
                nc.vector.tensor_tensor(u_, m_, wy, AL.mult)
                nc.vector.tensor_tensor(t_, m_, u_, AL.subtract)
                nc.vector.tensor_tensor(qcc[:, 3, :], u_, wx, AL.mult)
                nc.vector.tensor_tensor(qcc[:, 2, :], u_, qcc[:, 3, :],
                                        AL.subtract)
                nc.vector.tensor_tensor(qcc[:, 1, :], t_, wx, AL.mult)
                nc.vector.tensor_tensor(qcc[:, 0, :], t_, qcc[:, 1, :],
                                        AL.subtract)
                for j4 in range(4):
                    nc.scalar.dma_start(out=d_cm[j4, :, sl],
                                        in_=qcc[:, j4, :])
                nc.scalar.dma_start(
                    out=d_iw[:, :, ch * 64:(ch + 1) * 64],
                    in_=idx16[:, :, ch * 64:(ch + 1) * 64])


        # ================= Phase C: gather + hadamard + einsum =================
        with tc.tile_pool(name="cbp", bufs=3) as cbp, \
             tc.tile_pool(name="vp", bufs=2) as vp, \
             tc.tile_pool(name="pp", bufs=3) as ppool, \
             tc.tile_pool(name="psum_c", bufs=1, space="PSUM") as psc, \
             tc.tile_pool(name="outp", bufs=1) as outp:
            pos = psc.tile([P, S], F32)
            fflat = fpadA[:].rearrange("p a b -> p (a b)")
            # idx layout [p, k, hf, corner, 128]: a half's 4 corner lists are
            # contiguous so one 4-corner, 8192-idx gather covers them (the
            # cost model charges max(out_free, num_elems) per gather, so
            # batch the output well past the 5360-element source scan).
            HS = S // 2
            idx_prep(0)
            idx_prep(1)
            for hf in range(2):
                for k in range(9):
                    v = vp.tile([P, 2 * S], F32, tag="v")
                    nc.gpsimd.ap_gather(
                        out_ap=v[:], in_ap=fflat,
                        idxs_ap=idxall[:, k, hf].rearrange("p a b -> p (a b)"),
                        channels=P, num_elems=NELEM, d=1, num_idxs=2 * S)
                    for j4 in range(4):
                        cb = cbp.tile([P, HS], BF16, tag="cb")
                        nc.sync.dma_start(
                            out=cb[:],
                            in_=d_cm[j4, 2 * k: 2 * k + 2, None,
                                     hf * HS:(hf + 1) * HS]
                                .to_broadcast([2, 64, HS]))
                        pt = ppool.tile([P, HS], BF16, tag="pt")
                        nc.vector.tensor_tensor(
                            pt[:], v[:, j4 * HS:(j4 + 1) * HS], cb[:], AL.mult)
                        for b4 in range(4):
                            nc.tensor.matmul(
                                pos[:, hf * HS + b4 * 512:
                                    hf * HS + (b4 + 1) * 512],
                                ldcn[:, k, :],
                                pt[:, b4 * 512:(b4 + 1) * 512],
                                start=(k == 0 and j4 == 0),
                                stop=(k == 8 and j4 == 3))

            # out stage: bn3 + relu, un-permute i-order -> (y, x)
            ob = outp.tile([P, S], F32)
            obp = ob[:].rearrange("p (a b x) -> p b x a", a=16, b=2)
            posp = pos[:].rearrange("p (b x a) -> p b x a", b=2, x=128)
            ob3 = ob[:].rearrange("p (y x) -> p y x", y=QROWS)
            for oh in range(2):
                nc.scalar.activation(obp[:, oh], posp[:, oh],
                                     AF.Relu, bias=beta3, scale=inv3)
                nc.sync.dma_start(out=d_out[:, oh::2, :],
                                  in_=ob3[:, oh::2, :])

    nc.compile()
    return nc


_CACHE = {}


def _prep(inputs):
    f = {k: _f(v) for k, v in inputs.items()}
    inv1 = f['g1'] / np.sqrt(f['v1'] + EPS)
    beta1 = f['b1'] - f['m1'] * inv1
    inv2 = f['g2'] / np.sqrt(f['v2'] + EPS)
    beta2 = f['b2'] - f['m2'] * inv2
    invd = f['gd'] / np.sqrt(f['vd'] + EPS)
    betad = f['bd'] - f['md'] * invd
    inv3 = f['g3'] / np.sqrt(f['v3'] + EPS)
    beta3 = f['b3'] - f['m3'] * inv3

    lhsT1 = np.transpose(f['w1'], (1, 2, 3, 0)).reshape(Ci, 9, P)
    lhsT2 = np.transpose(f['w2'], (1, 2, 3, 0)).reshape(P, 9, P)
    wd = f['wd'][:, :, 0, 0] * (invd / inv2)[:, None]
    lhsT_sc = np.ascontiguousarray(wd.T)

    # offset conv rows: quant*18 + k*2 + d  <-  orig quant*18 + d*9 + k
    perm = np.zeros(54, dtype=np.int64)
    for quant in range(3):
        for kk in range(9):
            for dd in range(2):
                perm[quant * 18 + kk * 2 + dd] = quant * 18 + dd * 9 + kk
    ow = f['off_w'][perm]
    obias = f['off_b'][perm]
    lhsT_off = np.transpose(ow, (1, 2, 3, 0)).reshape(P, 9, 54)

    wr = f['dcn_w'].reshape(Co, DG, Cg, 9)
    lhsT_dcn = np.transpose(wr, (1, 2, 3, 0)).reshape(P, 9, Co)

    cst = np.zeros((P, 12), dtype=np.float32)
    cst[:, 9], cst[:, 10] = MAGIC, -MAGIC
    cst[:, 0], cst[:, 1] = inv1, beta1
    cst[:, 2], cst[:, 3] = inv2, beta2 + betad
    cst[:, 4], cst[:, 5] = inv3, beta3 + inv3 * f['dcn_b']
    for kd in range(18):
        cst[kd, 6] = obias[0 * 18 + kd] + 1.5   # dy bias + pad - 0.5
        cst[kd, 7] = obias[1 * 18 + kd] + 1.5   # dx bias + pad - 0.5
        cst[kd, 8] = obias[2 * 18 + kd]   # mask bias

    # i-order position constants: i = ((y%2)*128 + x)*16 + y//2
    ii = np.arange(S)
    aa = ii % 16
    cc = ii // 16
    bb2 = cc // 128
    xx = cc % 128
    yloc = 2 * aa + bb2
    y_add = np.zeros((18, S), dtype=np.float32)
    x_add = np.zeros((18, S), dtype=np.float32)
    for kk in range(9):
        for dd in range(2):
            kd = 2 * kk + dd
            y_add[kd] = yloc + (kk // 3)
            x_add[kd] = xx + (kk % 3)

    return dict(
        lhsT1=_f(lhsT1), lhsT2=_bf(lhsT2), lhsT_sc=_f(lhsT_sc),
        lhsT_off=_bf(lhsT_off), lhsT_dcn=_bf(lhsT_dcn),
        consts=_f(cst), y_add=_bf(y_add), x_add=_bf(x_add), x=f['x'])


def kernel(**inputs):
    cfg = _prep(inputs)
    x = cfg.pop('x')
    B = x.shape[0]

    if 'nc' not in _CACHE:
        _CACHE['nc'] = build_nc()
    nc = _CACHE['nc']

    in_maps = []
    for cid in range(8):
        b, q = cid // 4, cid % 4
        h0 = 32 * q
        xp = np.zeros((Ci, XR, XC), dtype=np.float32)
        r_lo = 2 * h0 - 9
        s_lo, s_hi = max(r_lo, 0), min(2 * h0 + 72, 256)
        xp[:, s_lo - r_lo: s_hi - r_lo, 1:257] = x[b, :, s_lo:s_hi, :]
        rm1 = np.zeros((P, F1R), dtype=np.float32)
        for f1 in range(F1R):
            rm1[:, f1] = 1.0 if 0 <= h0 - 4 + f1 < H else 0.0
        rmf = np.zeros((P, FR), dtype=np.float32)
        for f2 in range(38):
            rmf[:, f2] = 1.0 if 0 <= h0 - 3 + f2 < H else 0.0
        m = dict(cfg)
        m['x_shard'] = np.ascontiguousarray(xp)
        m['rowmask1'] = rm1
        m['rowmaskF'] = rmf
        in_maps.append(m)

    out = np.zeros((B, Co, H, W), dtype=np.float32)
    for attempt in range(2):
        res = run_bass_kernel_spmd(nc, in_maps, core_ids=list(range(8)))
        for cid in range(8):
            b, q = cid // 4, cid % 4
            out[b, :, 32 * q:32 * q + 32, :] = res.results[cid]['out']
        # a previously-wedged NeuronCore can surface one garbage run;
        # a clean retry recovers it.
        if np.isfinite(out).all():
            break
    return out
